# revision 4
# baseline (speedup 1.0000x reference)
"""Trainium2 Bass kernel for the LocalConnectivity diamond-ring stencil.

out[b, x, y] = sum_{1<=|dx|+|dy|<=5} w[|dx|+|dy|-1] * in[b, (x+dx)%512, (y+dy)%512]

Strategy (v2)
-------------
Data-parallel over batch: 64 samples -> 8 cores x 8 samples. Per sample the
512x512 grid is processed in 5 UNIFORM row-tiles of 103 output rows (the 5th
tile computes 3 extra wrapped rows that are simply not written back), so all
55 matmuls per sample share identical shapes and the same 11 banded weight
matrices.

The 60-tap stencil runs on the TensorEngine as 11 PSUM-accumulating matmuls
per tile, one per horizontal shift dy in [-5, 5]:

  psum[p, f] += sum_c  WB_dy[c, p] * X[c, f + dy_idx]

where X is the input tile with 5 halo rows on each side (contraction dim =
113 partitions) and 5 circular halo columns on each side, and WB_dy is the
banded Toeplitz matrix holding the vertical taps of kernel column dy.

v2 changes vs v1 (202 us):
 - Uniform tiles: one xt tile [128, 5, 522] per sample, 5 merged input DMAs
   (custom strided APs put the tile index in a free dim) instead of 7, and
   2 merged output DMAs instead of 5. Fewer software-DGE descriptor-build
   instructions on GpSimd (~1 us fixed cost each).
 - j-outer matmul order: the 5 tiles' matmuls for one dy run back-to-back
   against the same stationary matrix.
 - fp16 stationary weights (LDWEIGHTS at 2B/elem + FWL-class load) with
   f32r moving data. Weight quantization error ~5e-4 rel, well under
   tolerance.
 - Small wrap/halo DMAs moved to sync/scalar HW-DGE queues to keep GpSimd
   free for the bulk transfers.
 - PE stays continuously busy => HAM clock gate at 8/8 (2.4 GHz) instead of
   oscillating at 4/8.
"""

import numpy as np

import concourse.bass as bass
import concourse.bacc as bacc
import concourse.mybir as mybir
from concourse import tile
from concourse.bass_utils import run_bass_kernel_spmd

B, H, W = 64, 512, 512
NCORES = 8
BPC = B // NCORES  # samples per core
MAXD = 5
HALO = MAXD
DYS = 2 * MAXD + 1  # 11 horizontal shifts
TR = 103  # output rows per tile (uniform; tile 4 wraps, 3 rows discarded)
NT = 5
CTR = TR + 2 * HALO  # 113 contraction rows
XW = W + 2 * HALO  # 522

WEIGHT_DT = mybir.dt.float16  # stationary dtype
MOVING_DT = mybir.dt.float16  # moving dtype; input f32 cast to fp16 in the DMA


def _build_band_weights(dw: np.ndarray) -> np.ndarray:
    """[128, 11*128]: WB[c, j*128 + p] = K(c-p-5, j-5)."""
    wb = np.zeros((128, DYS, 128), dtype=np.float32)
    p = np.arange(128)
    for j in range(DYS):
        dy = j - MAXD
        for dx in range(-MAXD, MAXD + 1):
            d = abs(dx) + abs(dy)
            if 1 <= d <= MAXD:
                c = p + dx + HALO
                valid = (c >= 0) & (c < 128)
                wb[c[valid], j, p[valid]] = dw[d - 1]
    out = wb.reshape(128, DYS * 128)
    if WEIGHT_DT == mybir.dt.float16:
        out = out.astype(np.float16)
    return np.ascontiguousarray(out)


_CACHED_NC = None


def _custom_ap(base_ap, dims, extra_offset_elems=0):
    """Build a strided AP: dims = [(stride_elems, size), ...]."""
    s = base_ap.copy()
    s.ap.clear()
    s.ap.extend(dims)
    s.offset = s.offset + extra_offset_elems
    return s


def _build_program():
    f32 = mybir.dt.float32
    f32r = mybir.dt.float32r

    nc = bacc.Bacc(None, target_bir_lowering=False)
    x = nc.dram_tensor("x", [BPC, H, W], f32, kind="ExternalInput")
    wb = nc.dram_tensor("wb", [128, DYS * 128], WEIGHT_DT, kind="ExternalInput")
    y = nc.dram_tensor("y", [BPC, H, W], f32, kind="ExternalOutput")

    with tile.TileContext(nc) as tc:
        with (
            tc.tile_pool(name="wpool", bufs=1) as wpool,
            tc.tile_pool(name="xpool", bufs=3) as xpool,
            tc.tile_pool(name="opool", bufs=3) as opool,
            tc.tile_pool(name="pspool", bufs=8, space=bass.MemorySpace.PSUM) as pspool,
        ):
            wtile = wpool.tile([128, DYS * 128], WEIGHT_DT, tag="wt")
            nc.gpsimd.dma_start(wtile[:], wb[:])

            for b in range(BPC):
                # xt[p, t, 5+y] = x[b, (103*t - 5 + p) % 512, y]
                xt = xpool.tile([128, NT, XW], MOVING_DT, tag="xt")

                # bulk body rows for t=0..3: p=5..112 -> rows 103t + p - 5
                src_body = _custom_ap(
                    x[b], [(W, 108), (TR * W, 4), (1, W)]
                )
                nc.gpsimd.dma_start(xt[5:113, 0:4, HALO : HALO + W], src_body)
                # t=4 body: p=5..104 -> rows 412..511
                nc.gpsimd.dma_start(
                    xt[5:105, 4, HALO : HALO + W], x[b, 4 * TR : H, :]
                )
                # t=0 wrap-top: p=0..4 -> rows 507..511
                nc.gpsimd.dma_start(
                    xt[0:5, 0, HALO : HALO + W], x[b, H - HALO : H, :]
                )
                # halo tops t=1..4: p=0..4 -> rows 103t - 5 + p (base row 98)
                src_tops = _custom_ap(
                    x[b], [(W, 5), (TR * W, 4), (1, W)], extra_offset_elems=98 * W
                )
                nc.gpsimd.dma_start(xt[0:5, 1:5, HALO : HALO + W], src_tops)
                # t=4 wrap-bottom: p=105..112 -> rows 0..7
                nc.gpsimd.dma_start(
                    xt[105:113, 4, HALO : HALO + W], x[b, 0:8, :]
                )

                # circular column halos (on-chip copies)
                nc.scalar.copy(xt[0:113, :, 0:HALO], xt[0:113, :, W : W + HALO])
                nc.scalar.copy(
                    xt[0:113, :, HALO + W :], xt[0:113, :, HALO : 2 * HALO]
                )

                # ---- 55 matmuls: dy-outer so the stationary matrix is
                # reused across the 5 row-tiles ----
                pts = []
                for t in range(NT):
                    pt = pspool.tile([128, W], f32, tag="pt")
                    pts.append(pt)
                for j in range(DYS):
                    lhsT = wtile[0:CTR, j * 128 : j * 128 + TR]
                    for t in range(NT):
                        nc.tensor.matmul(
                            pts[t][0:TR, :],
                            lhsT,
                            xt[0:CTR, t, j : j + W],
                            start=(j == 0),
                            stop=(j == DYS - 1),
                        )

                # ---- PSUM eviction on VectorE ----
                otb = opool.tile([128, NT, W], f32, tag="otb")
                for t in range(NT):
                    nc.vector.tensor_copy(otb[0:TR, t, :], pts[t][0:TR, :])

                # ---- merged output DMAs ----
                # rows 0..411 from tiles 0..3
                dst_body = _custom_ap(
                    y[b], [(W, TR), (TR * W, 4), (1, W)]
                )
                nc.gpsimd.dma_start(dst_body, otb[0:TR, 0:4, :])
                # rows 412..511 from tile 4 (first 100 rows; wrapped dupes
                # in rows 100..102 are dropped)
                nc.gpsimd.dma_start(y[b, 4 * TR : H, :], otb[0:100, 4, :])
    nc.compile()
    return nc


def _get_program():
    global _CACHED_NC
    if _CACHED_NC is None:
        _CACHED_NC = _build_program()
    return _CACHED_NC


def _run(grid_spikes, distance_weights, trace=False):
    grid_spikes = np.ascontiguousarray(np.asarray(grid_spikes, dtype=np.float32))
    distance_weights = np.asarray(distance_weights, dtype=np.float32)
    assert grid_spikes.shape == (B, H, W), grid_spikes.shape
    wb_np = _build_band_weights(distance_weights)

    nc = _get_program()
    in_maps = [
        {
            "x": np.ascontiguousarray(grid_spikes[i * BPC : (i + 1) * BPC]),
            "wb": wb_np,
        }
        for i in range(NCORES)
    ]
    res = run_bass_kernel_spmd(nc, in_maps, list(range(NCORES)), trace=trace)
    out = np.concatenate([res.results[i]["y"] for i in range(NCORES)], axis=0)
    return out.astype(np.float32, copy=False), res


def kernel(grid_spikes, distance_weights):
    out, _ = _run(grid_spikes, distance_weights, trace=False)
    return out


def kernel_traced(grid_spikes, distance_weights):
    out, res = _run(grid_spikes, distance_weights, trace=True)
    return out, res
